# revision 15
# baseline (speedup 1.0000x reference)
"""FP8Linear Trainium2 kernel.

Computes out = quant_e4m3(x) @ quant_e4m3(w).T in fp32, distributed over 8
NeuronCores as a 2x4 grid (x rows x w rows). Per core:

  x_shard [4096, 2048] f32, w_shard [2048, 2048] f32 -> out [4096, 2048] f32

Per-core pipeline (v3):
  quantize: ACT f32 -> fp8e4 (exact quantize-dequantize grid).
  transpose to [c-part, c-chunk, *] layout via two engine routes:
    - PE route: gpsimd upcast fp8->bf16, PE identity-transpose (128x128,
      bf16, PSUM), gpsimd drain psum -> fp8 resident (downcast exact:
      values sit on the e4m3 grid).
    - DVE route: nc.vector.transpose strips ([32,512], 16 32x32 blocks
      per instruction, cross-quadrant at nch=32).
  matmul: fp8 DoubleRow (pairs of 128-deep c-chunks), fp32 PSUM, N=512.
  drains: ACT/gpsimd only (never DVE -- DVE is saturated with strips and
  would head-of-line block psum recycling). Stores ride the gpsimd queue.

Engine budget per core: PE ~ 240us matmul + its transpose share; DVE
strips ~ 0.7us per [32,512]; ACT quants ~ 60us + half drains; gpsimd
upcasts/drains/stores. Routing ratios are tunable via W_PE_CHUNKS /
X_PE_SUB.

TRN fp8e4 (max 240) matches OCP e4m3fn on [0, 240]; inputs are randn-scale
so the quantization grid is identical to the jax reference. Scales are
applied on the host (exact for any scale: round(x*s) then /(s_in*s_w)).
"""

import numpy as np
import ml_dtypes

# ---- problem constants (hardcoded per task contract) ----
A_DIM, B_DIM, C_DIM, OUT_DIM = 4, 2048, 2048, 8192
M_FULL = A_DIM * B_DIM  # 8192
GRID_M, GRID_O = 2, 4
N_CORES = GRID_M * GRID_O
M_CORE = M_FULL // GRID_M   # 4096
O_CORE = OUT_DIM // GRID_O  # 2048

P = 128
Q = 32  # stream-transpose square size

# routing knobs
W_PE_CHUNKS = 10   # of the 16 w o-chunks, how many transpose on PE
X_PE_SUB = 1       # default PE subchunks per x chunk (overridden per chunk)
X_PE_SUBS = [4, 2, 2, 2, 2, 1, 1, 1]  # per-chunk 128-row subchunks on PE


def build_nc(m_core=M_CORE, o_core=O_CORE, c_dim=C_DIM,
             m_chunk=512, n_tile=512, mm_psum_bufs=6,
             w_pe_chunks=W_PE_CHUNKS, x_pe_sub=X_PE_SUB):
    """Build the single-core Bass program (same program runs SPMD on 8 cores)."""
    import contextlib

    import concourse.bacc as bacc
    import concourse.mybir as mybir
    import concourse.tile as tile
    from concourse import masks

    f32 = mybir.dt.float32
    bf16 = mybir.dt.bfloat16
    fp8 = mybir.dt.float8e4
    Copy = mybir.ActivationFunctionType.Copy
    DR = mybir.MatmulPerfMode.DoubleRow

    S = c_dim // P              # c-chunks (16)
    SP = S // 2                 # DoubleRow pairs (8)
    MT = m_core // m_chunk      # x chunks (8)
    MW = m_chunk // P           # m windows per chunk (4)
    NT = o_core // n_tile       # o tiles (4)
    OC = o_core // P            # w o-chunks (16)
    OCN = OC // NT              # w o-chunks per n-tile (4)

    nc = bacc.Bacc(None, target_bir_lowering=False, debug=False)
    x_in = nc.declare_dram_parameter("x_in", [m_core, c_dim], f32, isOutput=False)
    w_in = nc.declare_dram_parameter("w_in", [o_core, c_dim], f32, isOutput=False)
    out = nc.declare_dram_parameter("out", [m_core, o_core], f32, isOutput=True)

    with tile.TileContext(nc) as tc:
        with contextlib.ExitStack() as ctx:
            const = ctx.enter_context(tc.tile_pool(name="const", bufs=1))
            wstg = ctx.enter_context(tc.tile_pool(name="wstg", bufs=4))
            w8p = ctx.enter_context(tc.tile_pool(name="w8p", bufs=4))
            wbp = ctx.enter_context(tc.tile_pool(name="wbp", bufs=2))
            wres = ctx.enter_context(tc.tile_pool(name="wres", bufs=1))
            tpp = ctx.enter_context(tc.tile_pool(name="tpp", bufs=2, space="PSUM"))
            xstg = ctx.enter_context(tc.tile_pool(name="xstg", bufs=3))
            x8p = ctx.enter_context(tc.tile_pool(name="x8p", bufs=8))
            xbp = ctx.enter_context(tc.tile_pool(name="xbp", bufs=2))
            xtf = ctx.enter_context(tc.tile_pool(name="xtf", bufs=4))
            mmp = ctx.enter_context(
                tc.tile_pool(name="mmp", bufs=mm_psum_bufs, space="PSUM"))
            osb = ctx.enter_context(tc.tile_pool(name="osb", bufs=4))

            identity = const.tile([P, P], bf16)
            masks.make_identity(nc, identity[:])
            ident8 = const.tile([P, P], fp8, tag="id8", name="id8")
            masks.make_identity(nc, ident8[:])

            # resident w.T as fp8, one tile per 512-o n-slice:
            WT = [wres.tile([P, S, n_tile], fp8, tag=f"WT{nt}", name=f"WT{nt}")
                  for nt in range(NT)]

            def strips(dst, src8, m_off):
                """16 DVE stream-transpose strips: src8 [128, S, 128]
                (rows r, c = 128u+32v+i) -> dst[32v+i, u, m_off+32g+j]."""
                for g in range(P // Q):
                    for v in range(P // Q):
                        nc.vector.transpose(
                            out=dst[Q * v:Q * (v + 1), :,
                                    m_off + Q * g:m_off + Q * (g + 1)],
                            in_=src8[Q * g:Q * (g + 1), :, Q * v:Q * (v + 1)])

            def pe_transpose(dst, src8, m_off, bpool, btag):
                """PE-route transpose of a 128-row fp8 tile: identity
                matmul in fp8 (ISA requires psum element step 2 for fp8
                transpose-mode, hence the trailing x2 dim), ACT drains
                psum -> fp8 dst."""
                for grp in range(S // 8):
                    pst = tpp.tile([P, 8, P, 2], fp8, tag="tp_ps",
                                   name="tp_ps")
                    for j in range(8):
                        u = 8 * grp + j
                        nc.tensor.transpose(pst[:, j, :, 0], src8[:, u, :],
                                            ident8[:])
                    # gpsimd cannot read PSUM; drain on ACT
                    nc.scalar.activation(
                        dst[:, 8 * grp:8 * grp + 8, m_off:m_off + P],
                        pst[:, :, :, 0], Copy)

            def w_load(oc):
                w_stage = wstg.tile([P, c_dim], f32, tag="w_stage",
                                    name="w_stage")
                # PE-route chunks ride the scalar queue, DVE-route the
                # gpsimd queue, so both transpose streams start immediately
                wq = nc.scalar if oc < w_pe_chunks else nc.gpsimd
                wq.dma_start(out=w_stage[:],
                             in_=w_in[oc * P:(oc + 1) * P, :])
                w8 = w8p.tile([P, S, P], fp8, tag="w8", name="w8")
                nc.scalar.activation(w8[:], w_stage[:], Copy)
                return w8

            def w_transpose(oc, w8, on_pe):
                dst = WT[oc // OCN]
                m_off = (oc % OCN) * P
                if on_pe:
                    pe_transpose(dst, w8, m_off, wbp, "wb16")
                else:
                    strips(dst, w8, m_off)

            def x_chain(mc):
                x_stage = xstg.tile([P, c_dim], f32, tag="x_stage",
                                    name="x_stage")
                nc.sync.dma_start(out=x_stage[:], in_=x_in[mc * P:(mc + 1) * P, :])
                x8 = x8p.tile([P, S, P], fp8, tag="x8", name="x8")
                nc.scalar.activation(x8[:], x_stage[:], Copy)
                return x8

            def x_transpose(mt, x8s, pe_subs):
                """Transpose the MW quantized row-chunks of chunk mt into an
                XT tile; the first pe_subs subchunks ride the PE route."""
                XT = xtf.tile([P, S, m_chunk], fp8, tag="XT", name="XT")
                for q, x8 in enumerate(x8s):
                    if q < pe_subs:
                        pe_transpose(XT, x8, q * P, xbp, "xb16")
                    else:
                        strips(XT, x8, q * P)
                return XT

            def mm(XT, ps, mw, sp, nt):
                nc.tensor.matmul(
                    ps[:],
                    XT[:, 2 * sp:2 * sp + 2, mw * P:(mw + 1) * P],
                    WT[nt][:, 2 * sp:2 * sp + 2, :],
                    start=(sp == 0), stop=(sp == SP - 1),
                    perf_mode=DR)

            def drain(ps, ot, nt, late=False):
                """psum -> full-row ot slice; ACT normally (gpsimd cannot
                read PSUM; DVE is saturated with strips until the last
                waves, where it takes the odd n-slices)."""
                dst = ot[:, nt * n_tile:(nt + 1) * n_tile]
                if late and nt % 2 == 1:
                    nc.vector.tensor_copy(out=dst, in_=ps[:])
                else:
                    nc.scalar.activation(dst, ps[:], Copy)

            def store(ot, m0, mw):
                nc.gpsimd.dma_start(
                    out=out[m0 + mw * P:m0 + (mw + 1) * P, :], in_=ot[:])

            def x_matmul_ntouter(mt, XT):
                """nt-outer: each n-slice needs only its own WT tile, so
                wave 0 can start before the DVE-routed WT halves land."""
                m0 = mt * m_chunk
                ots = [osb.tile([P, o_core], f32, tag="ot", name="ot")
                       for _ in range(MW)]
                for nt in range(NT):
                    for mw in range(MW):
                        ps = mmp.tile([P, n_tile], f32, tag="mm_psum",
                                      name="mm_psum")
                        for sp in range(SP):
                            mm(XT, ps, mw, sp, nt)
                        drain(ps, ots[mw], nt)
                        if nt == NT - 1:
                            store(ots[mw], m0, mw)

            def x_matmul(mt, XT):
                m0 = mt * m_chunk
                for mw in range(MW):
                    ps_tiles = [
                        mmp.tile([P, n_tile], f32, tag="mm_psum",
                                 name="mm_psum")
                        for _ in range(NT)]
                    for sp in range(SP):
                        for nt in range(NT):
                            mm(XT, ps_tiles[nt], mw, sp, nt)
                    ot = osb.tile([P, o_core], f32, tag="ot", name="ot")
                    for nt in range(NT):
                        drain(ps_tiles[nt], ot, nt)
                    store(ot, m0, mw)

            # ---- prep: all w chunks first (PE-routed ocs 0..w_pe-1 keep
            # the PE stream busy; DVE-routed ocs lead the DVE FIFO so WT
            # completes before x strips queue up). Then the first NT x
            # chunks; chunk 0 rides the PE route entirely so wave 0 can
            # start while the DVE is still chewing strips. ----
            x8s = {}
            xts = {}
            # prep: alternate PE-route and DVE-route w chunks so both
            # transpose streams start on their first chunk; the two routes
            # load on different DMA queues
            order = []
            for i in range(max(w_pe_chunks, OC - w_pe_chunks)):
                if i < w_pe_chunks:
                    order.append(i)
                if w_pe_chunks + i < OC:
                    order.append(w_pe_chunks + i)
            for oc in order:
                w8 = w_load(oc)
                w_transpose(oc, w8, oc < w_pe_chunks)
            # chunk 0, then wave 0 immediately (nt-outer), then the
            # remaining early x chunks
            x8s[0] = [x_chain(q) for q in range(MW)]
            xts[0] = x_transpose(0, x8s.pop(0), X_PE_SUBS[0])
            x_matmul_ntouter(0, xts.pop(0))
            for mt in range(1, NT):
                x8s[mt] = [x_chain(mt * MW + q) for q in range(MW)]
                xts[mt] = x_transpose(mt, x8s.pop(mt), X_PE_SUBS[mt])

            # ---- matmul waves ----
            for mt in range(MT):
                if mt > 0:
                    x_matmul(mt, xts.pop(mt))
                if mt + NT < MT:
                    x8s[mt + NT] = [x_chain((mt + NT) * MW + q)
                                    for q in range(MW)]
                nxt = mt + 2
                if nxt in x8s:
                    xts[nxt] = x_transpose(nxt, x8s.pop(nxt), X_PE_SUBS[nxt])

    nc.finalize()
    return nc


_NC = None


def _get_nc():
    global _NC
    if _NC is None:
        _NC = build_nc()
    return _NC


def kernel(input, weight, input_scale_e4m3=None, weight_scale_e4m3=None,
           **_unused):
    from concourse.bass_utils import run_bass_kernel_spmd

    x = np.asarray(input, dtype=np.float32).reshape(M_FULL, C_DIM)
    w = np.asarray(weight, dtype=np.float32)
    s_in = float(np.asarray(input_scale_e4m3)) if input_scale_e4m3 is not None else 1.0
    s_w = float(np.asarray(weight_scale_e4m3)) if weight_scale_e4m3 is not None else 1.0

    # reference semantics: round(x*s)/s etc.; fold scales on host (exact)
    if s_in != 1.0:
        x = x * s_in
    if s_w != 1.0:
        w = w * s_w

    nc = _get_nc()
    in_maps = []
    for mi in range(GRID_M):
        for oj in range(GRID_O):
            in_maps.append({
                "x_in": x[mi * M_CORE:(mi + 1) * M_CORE],
                "w_in": w[oj * O_CORE:(oj + 1) * O_CORE],
            })
    res = run_bass_kernel_spmd(nc, in_maps, core_ids=list(range(N_CORES)))

    out = np.empty((M_FULL, OUT_DIM), np.float32)
    for k, r in enumerate(res.results):
        mi, oj = divmod(k, GRID_O)
        out[mi * M_CORE:(mi + 1) * M_CORE, oj * O_CORE:(oj + 1) * O_CORE] = r["out"]

    inv = 1.0 / (s_in * s_w)
    if inv != 1.0:
        out = out * inv
    return out.reshape(A_DIM, B_DIM, OUT_DIM)


# revision 16
# speedup vs baseline: 1.1144x; 1.1144x over previous
"""FP8Linear Trainium2 kernel.

Computes out = quant_e4m3(x) @ quant_e4m3(w).T in fp32, distributed over 8
NeuronCores as a 2x4 grid (x rows x w rows). Per core:

  x_shard [4096, 2048] f32, w_shard [2048, 2048] f32 -> out [4096, 2048] f32

Per-core pipeline (v3):
  quantize: ACT f32 -> fp8e4 (exact quantize-dequantize grid).
  transpose to [c-part, c-chunk, *] layout via two engine routes:
    - PE route: fp8 identity-transpose (128x128; ISA requires psum
      element step 2 for fp8 transpose-mode), ACT drains psum -> fp8.
    - DVE route: nc.vector.transpose strips ([32,512], 16 32x32 blocks
      per instruction, cross-quadrant at nch=32), ~0.70us each.
  matmul: fp8 DoubleRow (pairs of 128-deep c-chunks), fp32 PSUM, N=512.
  psum drains: ACT only (gpsimd cannot read PSUM; DVE is saturated with
  strips and would head-of-line block psum recycling). Stores ride the
  gpsimd queue; w loads split across the scalar/gpsimd queues by route.

Engine balance per core (~24 transpose units each side): PE ~ 190us
matmul + ~3.2us per 128-row transpose unit; DVE ~ 11.2us per unit; ACT
quants + drains ~ 210us. Ratios tunable via W_PE_CHUNKS / X_PE_SUBS.

TRN fp8e4 (max 240) matches OCP e4m3fn on [0, 240]; inputs are randn-scale
so the quantization grid is identical to the jax reference. Scales are
applied on the host (exact for any scale: round(x*s) then /(s_in*s_w)).
"""

import numpy as np
import ml_dtypes

# ---- problem constants (hardcoded per task contract) ----
A_DIM, B_DIM, C_DIM, OUT_DIM = 4, 2048, 2048, 8192
M_FULL = A_DIM * B_DIM  # 8192
GRID_M, GRID_O = 2, 4
N_CORES = GRID_M * GRID_O
M_CORE = M_FULL // GRID_M   # 4096
O_CORE = OUT_DIM // GRID_O  # 2048

P = 128
Q = 32  # stream-transpose square size

# routing knobs
W_PE_CHUNKS = 10   # of the 16 w o-chunks, how many transpose on PE
X_PE_SUB = 1       # default PE subchunks per x chunk (overridden per chunk)
X_PE_SUBS = [4, 2, 2, 2, 2, 1, 1, 1]  # per-chunk 128-row subchunks on PE


def build_nc(m_core=M_CORE, o_core=O_CORE, c_dim=C_DIM,
             m_chunk=512, n_tile=512, mm_psum_bufs=6,
             w_pe_chunks=W_PE_CHUNKS, x_pe_sub=X_PE_SUB):
    """Build the single-core Bass program (same program runs SPMD on 8 cores)."""
    import contextlib

    import concourse.bacc as bacc
    import concourse.mybir as mybir
    import concourse.tile as tile
    from concourse import masks

    f32 = mybir.dt.float32
    bf16 = mybir.dt.bfloat16
    fp8 = mybir.dt.float8e4
    Copy = mybir.ActivationFunctionType.Copy
    DR = mybir.MatmulPerfMode.DoubleRow

    S = c_dim // P              # c-chunks (16)
    SP = S // 2                 # DoubleRow pairs (8)
    MT = m_core // m_chunk      # x chunks (8)
    MW = m_chunk // P           # m windows per chunk (4)
    NT = o_core // n_tile       # o tiles (4)
    OC = o_core // P            # w o-chunks (16)
    OCN = OC // NT              # w o-chunks per n-tile (4)

    nc = bacc.Bacc(None, target_bir_lowering=False, debug=False)
    x_in = nc.declare_dram_parameter("x_in", [m_core, c_dim], f32, isOutput=False)
    w_in = nc.declare_dram_parameter("w_in", [o_core, c_dim], f32, isOutput=False)
    out = nc.declare_dram_parameter("out", [m_core, o_core], f32, isOutput=True)

    with tile.TileContext(nc) as tc:
        with contextlib.ExitStack() as ctx:
            const = ctx.enter_context(tc.tile_pool(name="const", bufs=1))
            wstg = ctx.enter_context(tc.tile_pool(name="wstg", bufs=4))
            w8p = ctx.enter_context(tc.tile_pool(name="w8p", bufs=4))
            wbp = ctx.enter_context(tc.tile_pool(name="wbp", bufs=2))
            wres = ctx.enter_context(tc.tile_pool(name="wres", bufs=1))
            tpp = ctx.enter_context(tc.tile_pool(name="tpp", bufs=2, space="PSUM"))
            xstg = ctx.enter_context(tc.tile_pool(name="xstg", bufs=3))
            x8p = ctx.enter_context(tc.tile_pool(name="x8p", bufs=8))
            xbp = ctx.enter_context(tc.tile_pool(name="xbp", bufs=2))
            xtf = ctx.enter_context(tc.tile_pool(name="xtf", bufs=4))
            mmp = ctx.enter_context(
                tc.tile_pool(name="mmp", bufs=mm_psum_bufs, space="PSUM"))
            osb = ctx.enter_context(tc.tile_pool(name="osb", bufs=4))

            identity = const.tile([P, P], bf16)
            masks.make_identity(nc, identity[:])
            ident8 = const.tile([P, P], fp8, tag="id8", name="id8")
            masks.make_identity(nc, ident8[:])

            # resident w.T as fp8, one tile per 512-o n-slice:
            WT = [wres.tile([P, S, n_tile], fp8, tag=f"WT{nt}", name=f"WT{nt}")
                  for nt in range(NT)]

            def strips(dst, src8, m_off):
                """16 DVE stream-transpose strips: src8 [128, S, 128]
                (rows r, c = 128u+32v+i) -> dst[32v+i, u, m_off+32g+j]."""
                for g in range(P // Q):
                    for v in range(P // Q):
                        nc.vector.transpose(
                            out=dst[Q * v:Q * (v + 1), :,
                                    m_off + Q * g:m_off + Q * (g + 1)],
                            in_=src8[Q * g:Q * (g + 1), :, Q * v:Q * (v + 1)])

            def pe_transpose(dst, src8, m_off, bpool, btag):
                """PE-route transpose of a 128-row fp8 tile: identity
                matmul in fp8 (ISA requires psum element step 2 for fp8
                transpose-mode, hence the trailing x2 dim), ACT drains
                psum -> fp8 dst."""
                for grp in range(S // 8):
                    pst = tpp.tile([P, 8, P, 2], fp8, tag="tp_ps",
                                   name="tp_ps")
                    for j in range(8):
                        u = 8 * grp + j
                        nc.tensor.transpose(pst[:, j, :, 0], src8[:, u, :],
                                            ident8[:])
                    # gpsimd cannot read PSUM; drain on ACT
                    nc.scalar.activation(
                        dst[:, 8 * grp:8 * grp + 8, m_off:m_off + P],
                        pst[:, :, :, 0], Copy)

            def w_load(oc):
                w_stage = wstg.tile([P, c_dim], f32, tag="w_stage",
                                    name="w_stage")
                # PE-route chunks ride the scalar queue, DVE-route the
                # gpsimd queue, so both transpose streams start immediately
                wq = nc.scalar if oc < w_pe_chunks else nc.gpsimd
                wq.dma_start(out=w_stage[:],
                             in_=w_in[oc * P:(oc + 1) * P, :])
                w8 = w8p.tile([P, S, P], fp8, tag="w8", name="w8")
                nc.scalar.activation(w8[:], w_stage[:], Copy)
                return w8

            def w_transpose(oc, w8, on_pe):
                dst = WT[oc // OCN]
                m_off = (oc % OCN) * P
                if on_pe:
                    pe_transpose(dst, w8, m_off, wbp, "wb16")
                else:
                    strips(dst, w8, m_off)

            def x_chain(mc):
                x_stage = xstg.tile([P, c_dim], f32, tag="x_stage",
                                    name="x_stage")
                nc.sync.dma_start(out=x_stage[:], in_=x_in[mc * P:(mc + 1) * P, :])
                x8 = x8p.tile([P, S, P], fp8, tag="x8", name="x8")
                nc.scalar.activation(x8[:], x_stage[:], Copy)
                return x8

            def x_transpose(mt, x8s, pe_subs):
                """Transpose the MW quantized row-chunks of chunk mt into an
                XT tile; the first pe_subs subchunks ride the PE route."""
                XT = xtf.tile([P, S, m_chunk], fp8, tag="XT", name="XT")
                for q, x8 in enumerate(x8s):
                    if q < pe_subs:
                        pe_transpose(XT, x8, q * P, xbp, "xb16")
                    else:
                        strips(XT, x8, q * P)
                return XT

            def mm(XT, ps, mw, sp, nt):
                nc.tensor.matmul(
                    ps[:],
                    XT[:, 2 * sp:2 * sp + 2, mw * P:(mw + 1) * P],
                    WT[nt][:, 2 * sp:2 * sp + 2, :],
                    start=(sp == 0), stop=(sp == SP - 1),
                    perf_mode=DR)

            def drain(ps, ot, nt, late=False):
                """psum -> full-row ot slice; ACT normally (gpsimd cannot
                read PSUM; DVE is saturated with strips until the last
                waves, where it takes the odd n-slices)."""
                dst = ot[:, nt * n_tile:(nt + 1) * n_tile]
                if late and nt % 2 == 1:
                    nc.vector.tensor_copy(out=dst, in_=ps[:])
                else:
                    nc.scalar.activation(dst, ps[:], Copy)

            def store(ot, m0, mw):
                nc.gpsimd.dma_start(
                    out=out[m0 + mw * P:m0 + (mw + 1) * P, :], in_=ot[:])

            def x_matmul_ntouter(mt, XT):
                """nt-outer: each n-slice needs only its own WT tile, so
                wave 0 can start before the DVE-routed WT halves land."""
                m0 = mt * m_chunk
                ots = [osb.tile([P, o_core], f32, tag="ot", name="ot")
                       for _ in range(MW)]
                for nt in range(NT):
                    for mw in range(MW):
                        ps = mmp.tile([P, n_tile], f32, tag="mm_psum",
                                      name="mm_psum")
                        for sp in range(SP):
                            mm(XT, ps, mw, sp, nt)
                        drain(ps, ots[mw], nt)
                        if nt == NT - 1:
                            store(ots[mw], m0, mw)

            def x_matmul(mt, XT):
                m0 = mt * m_chunk
                for mw in range(MW):
                    ps_tiles = [
                        mmp.tile([P, n_tile], f32, tag="mm_psum",
                                 name="mm_psum")
                        for _ in range(NT)]
                    for sp in range(SP):
                        for nt in range(NT):
                            mm(XT, ps_tiles[nt], mw, sp, nt)
                    ot = osb.tile([P, o_core], f32, tag="ot", name="ot")
                    for nt in range(NT):
                        drain(ps_tiles[nt], ot, nt)
                    store(ot, m0, mw)

            # ---- prep: all w chunks first (PE-routed ocs 0..w_pe-1 keep
            # the PE stream busy; DVE-routed ocs lead the DVE FIFO so WT
            # completes before x strips queue up). Then the first NT x
            # chunks; chunk 0 rides the PE route entirely so wave 0 can
            # start while the DVE is still chewing strips. ----
            x8s = {}
            xts = {}
            # prep: alternate PE-route and DVE-route w chunks so both
            # transpose streams start on their first chunk; the two routes
            # load on different DMA queues
            order = []
            for i in range(max(w_pe_chunks, OC - w_pe_chunks)):
                if i < w_pe_chunks:
                    order.append(i)
                if w_pe_chunks + i < OC:
                    order.append(w_pe_chunks + i)
            for oc in order:
                w8 = w_load(oc)
                w_transpose(oc, w8, oc < w_pe_chunks)
            # chunk 0, then wave 0 immediately (nt-outer), then the
            # remaining early x chunks
            x8s[0] = [x_chain(q) for q in range(MW)]
            xts[0] = x_transpose(0, x8s.pop(0), X_PE_SUBS[0])
            x_matmul_ntouter(0, xts.pop(0))
            for mt in range(1, NT):
                x8s[mt] = [x_chain(mt * MW + q) for q in range(MW)]
                xts[mt] = x_transpose(mt, x8s.pop(mt), X_PE_SUBS[mt])

            # ---- matmul waves ----
            for mt in range(MT):
                if mt > 0:
                    x_matmul(mt, xts.pop(mt))
                if mt + NT < MT:
                    x8s[mt + NT] = [x_chain((mt + NT) * MW + q)
                                    for q in range(MW)]
                nxt = mt + 2
                if nxt in x8s:
                    xts[nxt] = x_transpose(nxt, x8s.pop(nxt), X_PE_SUBS[nxt])

    nc.finalize()
    return nc


_NC = None


def _get_nc():
    global _NC
    if _NC is None:
        _NC = build_nc()
    return _NC


def kernel(input, weight, input_scale_e4m3=None, weight_scale_e4m3=None,
           **_unused):
    from concourse.bass_utils import run_bass_kernel_spmd

    x = np.asarray(input, dtype=np.float32).reshape(M_FULL, C_DIM)
    w = np.asarray(weight, dtype=np.float32)
    s_in = float(np.asarray(input_scale_e4m3)) if input_scale_e4m3 is not None else 1.0
    s_w = float(np.asarray(weight_scale_e4m3)) if weight_scale_e4m3 is not None else 1.0

    # reference semantics: round(x*s)/s etc.; fold scales on host (exact)
    if s_in != 1.0:
        x = x * s_in
    if s_w != 1.0:
        w = w * s_w

    nc = _get_nc()
    in_maps = []
    for mi in range(GRID_M):
        for oj in range(GRID_O):
            in_maps.append({
                "x_in": x[mi * M_CORE:(mi + 1) * M_CORE],
                "w_in": w[oj * O_CORE:(oj + 1) * O_CORE],
            })
    res = run_bass_kernel_spmd(nc, in_maps, core_ids=list(range(N_CORES)))

    out = np.empty((M_FULL, OUT_DIM), np.float32)
    for k, r in enumerate(res.results):
        mi, oj = divmod(k, GRID_O)
        out[mi * M_CORE:(mi + 1) * M_CORE, oj * O_CORE:(oj + 1) * O_CORE] = r["out"]

    inv = 1.0 / (s_in * s_w)
    if inv != 1.0:
        out = out * inv
    return out.reshape(A_DIM, B_DIM, OUT_DIM)
